# revision 1
# baseline (speedup 1.0000x reference)
"""GNN message passing + 3x conv3x3 + leaky-relu, distributed over 8 trn2 NeuronCores.

Strategy (node-sharded, 128 nodes/core):
- Pooling (pos/neg masked bidirectional scatter-add) is done entirely by SWDGE
  indirect-gather DMAs with compute_op=add: the feats table is laid out as
  (node, channel) rows of 4 KiB; each gather round pulls 128 rows (4 nodes x
  2 signs x 16 ch) and accumulates into an SBUF tile that is ALREADY in
  conv layout (partition=channel). Zero compute-engine cost, exact fp32.
- Convs run as 9 shifted-tap matmuls over a 34x34 zero-padded grid using
  strided access-pattern views (no im2col), bf16 operands, fp32 PSUM
  accumulation. Four nodes are packed per 128-partition bundle and mapped to
  disjoint 32x32 PE sub-array rectangles via tile_position, so the 128x128
  array stays busy despite 32-channel convs.
- leaky_relu(x) = x + relu(-0.9 x): one ScalarE activation + one VectorE
  tensor_tensor add per bundle, full 128-lane ops.
"""

import numpy as np

N, C, H, W = 1024, 16, 32, 32
NCORES = 8
NPC = N // NCORES            # nodes per core
GROUPS = NPC // 4            # acc groups of 4 nodes per core
CONV_ROUNDS = NPC // 16      # 16 nodes per conv round (4 bundles)
HP = WP = H + 2
GRID = HP * WP
HW = H * W
ZROW = 16 * N                # first zero row in the gather table

_prog_cache = {}


def _make_tile_context(nc):
    """TileContext whose lowering splits multi-sem waits onto nop carriers
    (this walrus build accepts at most one sync wait per instruction) and
    whose tail drain does the same."""
    import concourse.mybir as mybir
    import concourse.tile as tile

    class _TC(tile.TileContext):
        def _lower_ordered_insts(self, ordered):
            for bb_name, insts in ordered.items():
                out = []
                for inst in insts:
                    si = inst.sync_info
                    waits = list(si.on_wait) if si is not None and si.on_wait else []
                    if len(waits) > 1:
                        for w in waits[:-1]:
                            car = mybir.InstNoOp(
                                name=self.nc.get_next_instruction_name(),
                                ins=[], outs=[])
                            car.engine = inst.engine
                            car.sync_info = mybir.SyncInfo(on_wait=[w], on_update=[])
                            self.nc.register_instruction(car, overwrite=True)
                            out.append(car)
                        inst.sync_info = mybir.SyncInfo(
                            on_wait=[waits[-1]],
                            on_update=list(si.on_update) if si.on_update else [])
                    out.append(inst)
                insts[:] = out
            return super()._lower_ordered_insts(ordered)

        def _drain_and_barrier(self, tick_clock, wait_clock):
            clock = tick_clock.global_clock
            allocated = wait_clock.sems.allocated()
            for proc, tick in enumerate(clock):
                if tick > 0 and proc in allocated:
                    n = self.nc.sync.nop(nofuse=True, hint="tailwait")
                    n.wait_op(allocated[proc], tick, "sem-ge")
            self.nc.sync.drain()
            self.nc.all_engine_barrier()
            assert self.sems is not None
            popped = self.nc._tile_sem_poison_stack.pop()
            assert popped is self._sem_poison
            self.nc.clear_and_free_semaphores(list(self.sems.allocated().values()))
            self.nc.all_engine_barrier()

    return _TC(nc)


def _build_program(r_list, r_off, r_total, with_bias, variant="full"):
    import os
    import concourse.bass as bass
    import concourse.mybir as mybir

    reps = 1
    if "x" in variant:
        variant, _, r = variant.partition("x")
        reps = int(r)
    do_gather = variant in ("full", "gather")
    do_conv = variant in ("full", "conv")

    f32 = mybir.dt.float32
    bf16 = mybir.dt.bfloat16
    i32 = mybir.dt.int32
    AF = mybir.ActivationFunctionType
    ALU = mybir.AluOpType

    nc = bass.Bass()
    tab_d = nc.dram_tensor("tab", [16 * N + 16, HW], f32, kind="ExternalInput")
    gidx_d = nc.dram_tensor("gidx", [128, r_total], i32, kind="ExternalInput")
    fown_d = nc.dram_tensor("fown", [NPC * C, HW], f32, kind="ExternalInput")
    w1pn_d = nc.dram_tensor("w1pn", [128, 9 * 32], bf16, kind="ExternalInput")
    w1s_d = nc.dram_tensor("w1s", [128, 9 * 32], bf16, kind="ExternalInput")
    w2_d = nc.dram_tensor("w2", [128, 9 * 32], bf16, kind="ExternalInput")
    w3_d = nc.dram_tensor("w3", [128, 9 * 16], bf16, kind="ExternalInput")
    bias_d = nc.dram_tensor("bias", [128, 3], f32, kind="ExternalInput")
    y_d = nc.dram_tensor("y", [NPC * C, HW], f32, kind="ExternalOutput")

    def valid(ap_grid):
        # [p, GRID] tile AP -> [p, 32, 32] interior view of the 34x34 grid
        return ap_grid.rearrange("p (h w) -> p h w", w=WP)[:, 1:H + 1, 1:W + 1]

    def tap_view(ap_grid, base, k, dy, dx, h0):
        # rhs view for tap (dy,dx), output rows [h0, h0+16), K channels at
        # partition `base`
        g3 = ap_grid.rearrange("p (h w) -> p h w", w=WP)
        return g3[base:base + k, h0 + dy:h0 + dy + 16, dx:dx + W]

    tc = _make_tile_context(nc)
    with tc:
        with (tc.tile_pool(name="cw", bufs=1) as cw,
              tc.tile_pool(name="accp", bufs=6) as accp,
              tc.tile_pool(name="x1pnp", bufs=3) as x1pnp,
              tc.tile_pool(name="fop", bufs=3) as fop,
              tc.tile_pool(name="x1sp", bufs=3) as x1sp,
              tc.tile_pool(name="x2p", bufs=3) as x2p,
              tc.tile_pool(name="x3p", bufs=3) as x3p,
              tc.tile_pool(name="r2p", bufs=3) as r2p,
              tc.tile_pool(name="osbp", bufs=3) as osbp,
              tc.tile_pool(name="psp", bufs=4, space="PSUM") as psp):
            idx_t = cw.tile([128, r_total], i32)
            nc.sync.dma_start(out=idx_t[:], in_=gidx_d[:])
            w1pn_t = cw.tile([128, 9 * 32], bf16)
            nc.sync.dma_start(out=w1pn_t[:], in_=w1pn_d[:])
            w1s_t = cw.tile([128, 9 * 32], bf16)
            nc.sync.dma_start(out=w1s_t[:], in_=w1s_d[:])
            w2_t = cw.tile([128, 9 * 32], bf16)
            nc.sync.dma_start(out=w2_t[:], in_=w2_d[:])
            w3_t = cw.tile([128, 9 * 16], bf16)
            nc.sync.dma_start(out=w3_t[:], in_=w3_d[:])
            if with_bias:
                bias_t = cw.tile([128, 3], f32)
                nc.sync.dma_start(out=bias_t[:], in_=bias_d[:])

            memset_count = {}

            def fresh_grid(pool, name):
                t = pool.tile([128, GRID], bf16, tag=name)
                c = memset_count.get(name, 0)
                if c < 3:  # pool bufs
                    nc.vector.memset(t[:], 0.0)
                    memset_count[name] = c + 1
                return t

            def fresh_fo(pool, name):
                t = pool.tile([128, HW], bf16, tag=name)
                c = memset_count.get(name, 0)
                if c < 3:
                    nc.vector.memset(t[:], 0.0)
                    memset_count[name] = c + 1
                return t

            for rnd in [r for _ in range(reps) for r in range(CONV_ROUNDS)]:
                x1pn_tiles = []
                x1s_tiles = []
                for b in range(4):
                    g = 4 * rnd + b
                    # ---- pooling: accumulate gathers into acc (f32, exact)
                    acc_t = accp.tile([128, HW], f32, tag="acc")
                    if do_gather:
                        for r in range(r_list[g]):
                            col = r_off[g] + r
                            nc.gpsimd.indirect_dma_start(
                                out=acc_t[:], out_offset=None, in_=tab_d[:],
                                in_offset=bass.IndirectOffsetOnAxis(
                                    ap=idx_t[:, col:col + 1], axis=0),
                                compute_op=ALU.bypass if r == 0 else ALU.add)
                    else:
                        nc.vector.memset(acc_t[:], 0.0)
                    if not do_conv:
                        continue
                    # ---- X1 pos/neg grid (bf16, strided valid write)
                    x1 = fresh_grid(x1pnp, "x1pn")
                    nc.vector.tensor_copy(
                        out=valid(x1[:]),
                        in_=acc_t[:].rearrange("p (h w) -> p h w", w=W))
                    x1pn_tiles.append(x1)
                    # ---- X1 self grid: stage feats_own (cast bf16) then copy
                    fo = fresh_fo(fop, "fo")
                    for j in range(4):
                        slot = 16 * rnd + 4 * b + j
                        nc.gpsimd.dma_start(
                            out=fo[32 * j:32 * j + C, :],
                            in_=fown_d[C * slot:C * slot + C, :])
                    x1s = fresh_grid(x1sp, "x1s")
                    nc.vector.tensor_copy(
                        out=valid(x1s[:]),
                        in_=fo[:].rearrange("p (h w) -> p h w", w=W))
                    x1s_tiles.append(x1s)

                for b in range(4 if do_conv else 0):
                    x1, x1s = x1pn_tiles[b], x1s_tiles[b]
                    # ---- conv1: pass1 K=32 (pos+neg), pass2 K=16 (self)
                    ps1 = psp.tile([128, HW], f32, tag="ps")
                    ps1v = ps1[:].rearrange("p (h w) -> p h w", w=W)
                    for j in range(4):
                        cs = (j + b) % 4
                        for h0 in (0, 16):
                            for t in range(9):
                                dy, dx = t // 3, t % 3
                                nc.tensor.matmul(
                                    out=ps1v[32 * cs:32 * cs + 32, h0:h0 + 16, :],
                                    lhsT=w1pn_t[32 * j:32 * j + 32, t * 32:t * 32 + 32],
                                    rhs=tap_view(x1[:], 32 * j, 32, dy, dx, h0),
                                    start=(t == 0), stop=False,
                                    tile_position=(32 * j, 32 * cs))
                            for t in range(9):
                                dy, dx = t // 3, t % 3
                                nc.tensor.matmul(
                                    out=ps1v[32 * cs:32 * cs + 32, h0:h0 + 16, :],
                                    lhsT=w1s_t[32 * j:32 * j + 32, t * 32:t * 32 + 32],
                                    rhs=tap_view(x1s[:], 32 * j, 32, dy, dx, h0),
                                    start=False, stop=(t == 8),
                                    tile_position=(32 * j, 32 * cs))
                    r2a = r2p.tile([128, HW], bf16, tag="r2")
                    nc.scalar.activation(out=r2a[:], in_=ps1[:], func=AF.Relu,
                                         scale=-0.9)
                    x2 = fresh_grid(x2p, "x2")
                    nc.vector.tensor_tensor(
                        out=valid(x2[:]),
                        in0=ps1[:].rearrange("p (h w) -> p h w", w=W),
                        in1=r2a[:].rearrange("p (h w) -> p h w", w=W),
                        op=ALU.add)

                    # ---- conv2 (K=32)
                    ps2 = psp.tile([128, HW], f32, tag="ps")
                    ps2v = ps2[:].rearrange("p (h w) -> p h w", w=W)
                    for q in range(4):
                        cs = (q + b + 1) % 4
                        for h0 in (0, 16):
                            for t in range(9):
                                dy, dx = t // 3, t % 3
                                nc.tensor.matmul(
                                    out=ps2v[32 * cs:32 * cs + 32, h0:h0 + 16, :],
                                    lhsT=w2_t[32 * q:32 * q + 32, t * 32:t * 32 + 32],
                                    rhs=tap_view(x2[:], 32 * q, 32, dy, dx, h0),
                                    start=(t == 0), stop=(t == 8),
                                    tile_position=(32 * q, 32 * cs))
                    r2b = r2p.tile([128, HW], bf16, tag="r2")
                    nc.scalar.activation(out=r2b[:], in_=ps2[:], func=AF.Relu,
                                         scale=-0.9)
                    x3 = fresh_grid(x3p, "x3")
                    nc.vector.tensor_tensor(
                        out=valid(x3[:]),
                        in0=ps2[:].rearrange("p (h w) -> p h w", w=W),
                        in1=r2b[:].rearrange("p (h w) -> p h w", w=W),
                        op=ALU.add)

                    # ---- conv3 (K=32, M=16)
                    ps3 = psp.tile([128, HW], f32, tag="ps")
                    ps3v = ps3[:].rearrange("p (h w) -> p h w", w=W)
                    for q in range(4):
                        cs = (q + b + 2) % 4
                        for h0 in (0, 16):
                            for t in range(9):
                                dy, dx = t // 3, t % 3
                                nc.tensor.matmul(
                                    out=ps3v[32 * cs:32 * cs + 16, h0:h0 + 16, :],
                                    lhsT=w3_t[32 * q:32 * q + 32, t * 16:t * 16 + 16],
                                    rhs=tap_view(x3[:], 32 * q, 32, dy, dx, h0),
                                    start=(t == 0), stop=(t == 8),
                                    tile_position=(32 * q, 32 * cs))
                    r2c = r2p.tile([128, HW], bf16, tag="r2")
                    nc.scalar.activation(out=r2c[:], in_=ps3[:], func=AF.Relu,
                                         scale=-0.9)
                    osb = osbp.tile([128, HW], f32, tag="osb")
                    nc.vector.tensor_tensor(out=osb[:], in0=ps3[:], in1=r2c[:],
                                            op=ALU.add)
                    for j in range(4):
                        q1 = (j + b) % 4
                        q2 = (q1 + b + 1) % 4
                        q3 = (q2 + b + 2) % 4
                        slot = 16 * rnd + 4 * b + j
                        nc.sync.dma_start(
                            out=y_d[C * slot:C * slot + C, :],
                            in_=osb[32 * q3:32 * q3 + C, :])
    return nc


def _host_prep(feats, edges, w1, b1, w2, b2, w3, b3):
    import ml_dtypes

    feats = np.ascontiguousarray(np.asarray(feats, dtype=np.float32))
    edges = np.asarray(edges)
    w1 = np.asarray(w1, dtype=np.float32)
    w2 = np.asarray(w2, dtype=np.float32)
    w3 = np.asarray(w3, dtype=np.float32)

    # per-(node, sign) contribution lists
    contrib = [([], []) for _ in range(N)]
    for s, sg, d in edges.tolist():
        si = 0 if sg > 0 else 1
        contrib[d][si].append(s)
        contrib[s][si].append(d)

    # per-core slot ordering: sort by max degree so groups of 4 have similar
    # round counts (minimises padded gather rounds)
    slot2node = []
    for k in range(NCORES):
        nodes = list(range(NPC * k, NPC * (k + 1)))
        nodes.sort(key=lambda n: -max(len(contrib[n][0]), len(contrib[n][1])))
        slot2node.append(nodes)

    # group round counts, maxed across cores (program must be SPMD-uniform)
    r_list = []
    for g in range(GROUPS):
        r = 1
        for k in range(NCORES):
            for j in range(4):
                n = slot2node[k][4 * g + j]
                r = max(r, len(contrib[n][0]), len(contrib[n][1]))
        r_list.append(r)
    r_off = np.concatenate([[0], np.cumsum(r_list)]).astype(int)
    r_total = int(r_off[-1])

    feats2d = feats.reshape(N * C, HW)
    tab = np.concatenate([feats2d, np.zeros((C, HW), np.float32)], axis=0)

    # weight tiles (lhsT layout, replicated across the 4 row slots)
    def wtile(w, ci_lo, ci_n, co_n):
        t = np.zeros((128, 9 * co_n), np.float32)
        for rs in range(4):
            for tp in range(9):
                dy, dx = tp // 3, tp % 3
                t[32 * rs:32 * rs + ci_n, tp * co_n:(tp + 1) * co_n] = \
                    w[:, ci_lo:ci_lo + ci_n, dy, dx].T
        return t.astype(ml_dtypes.bfloat16)

    w1pn = wtile(w1, C, 2 * C, 2 * C)
    w1s = wtile(w1, 0, C, 2 * C)
    w2t = wtile(w2, 0, 2 * C, 2 * C)
    w3t = wtile(w3, 0, 2 * C, C)
    biases = np.zeros((128, 3), np.float32)

    in_maps = []
    chan = np.arange(128) % C
    for k in range(NCORES):
        gidx = np.empty((128, r_total), np.int32)
        gidx[:] = (ZROW + chan)[:, None]
        for g in range(GROUPS):
            for j in range(4):
                n = slot2node[k][4 * g + j]
                for si in range(2):
                    lst = contrib[n][si]
                    base = 32 * j + 16 * si
                    for r, m in enumerate(lst):
                        gidx[base:base + C, r_off[g] + r] = C * m + chan[:C]
        rows = np.concatenate(
            [np.arange(C * n, C * n + C) for n in slot2node[k]])
        fown = feats2d[rows]
        in_maps.append({
            "tab": tab, "gidx": gidx, "fown": np.ascontiguousarray(fown),
            "w1pn": w1pn, "w1s": w1s, "w2": w2t, "w3": w3t, "bias": biases,
        })
    return in_maps, slot2node, tuple(r_list), tuple(r_off[:-1].tolist()), r_total


def kernel(feats, edges, w1, b1, w2, b2, w3, b3):
    from concourse.bass_utils import run_bass_kernel_spmd

    in_maps, slot2node, r_list, r_off, r_total = _host_prep(
        feats, edges, w1, b1, w2, b2, w3, b3)
    with_bias = bool(np.any(np.asarray(b1)) or np.any(np.asarray(b2))
                     or np.any(np.asarray(b3)))
    assert not with_bias, "nonzero conv biases not implemented"

    key = (r_list, with_bias)
    nc = _prog_cache.get(key)
    if nc is None:
        nc = _build_program(r_list, r_off, r_total, with_bias)
        _prog_cache[key] = nc

    import os
    trace = bool(os.environ.get("KERNEL_TRACE"))
    res = run_bass_kernel_spmd(nc, in_maps, core_ids=list(range(NCORES)),
                               trace=trace)
    if trace:
        global last_results
        last_results = res

    out = np.empty((N, C, H, W), np.float32)
    for k in range(NCORES):
        yk = res.results[k]["y"]
        for i, n in enumerate(slot2node[k]):
            out[n] = yk[C * i:C * i + C].reshape(C, H, W)
    return out



# revision 14
# speedup vs baseline: 6.2740x; 6.2740x over previous
"""GNN message passing + 3x conv3x3 + leaky-relu, distributed over 8 trn2 NeuronCores.

Strategy v2 (node-sharded, 128 nodes/core):
- Pooling: host reorders neighbor features into per-round contiguous DRAM
  tables (one row per SBUF partition); the device accumulates them into a
  resident pooled mega-tile with a handful of large SWDGE DMAs using
  cce accumulate (bf16 source rows, f32 accumulation via dge-cast).
  Slots are degree-sorted so round r covers a contiguous suffix of node
  blocks -> zero indirection, ~64KB descriptors.
- Convs: 4 nodes packed per 128-partition bundle with BLOCK-DIAGONAL
  weights -> K=128 single matmuls instead of 4x (32x32) tile-position
  matmuls. 4x fewer PE instructions for the same math. 9 shifted-tap
  matmuls over a 34x34 zero-padded grid (no im2col), bf16 operands,
  fp32 PSUM accumulation.
- leaky_relu(x) = x + relu(-0.9 x): ScalarE activation + VectorE add.
- Bundles are consumed lowest-degree-first so conv overlaps the tail of
  the gather chain; pooled tensor is split into 4 sub-tiles so tile
  dependencies stay coarse but overlappable.
"""

import numpy as np

N, C, H, W = 1024, 16, 32, 32
NCORES = 8
NPC = N // NCORES            # nodes per core
NB = NPC // 4                # bundles (4 nodes) per core
NSUB = 4                     # pooled-accumulator sub-tiles
BPS = NB // NSUB             # bundles per sub-tile
HP = WP = H + 2
GRID = HP * WP
HW = H * W

GATHER_BF16 = False  # gather-table dtype; f32 accumulate either way

_prog_cache = {}


def _make_tile_context(nc):
    """TileContext whose lowering splits multi-sem waits onto nop carriers
    (this walrus build accepts at most one sync wait per instruction) and
    whose tail drain does the same."""
    import concourse.mybir as mybir
    import concourse.tile as tile

    class _TC(tile.TileContext):
        def _lower_ordered_insts(self, ordered):
            for bb_name, insts in ordered.items():
                out = []
                for inst in insts:
                    si = inst.sync_info
                    waits = list(si.on_wait) if si is not None and si.on_wait else []
                    if len(waits) > 1:
                        for w in waits[:-1]:
                            car = mybir.InstNoOp(
                                name=self.nc.get_next_instruction_name(),
                                ins=[], outs=[])
                            car.engine = inst.engine
                            car.sync_info = mybir.SyncInfo(on_wait=[w], on_update=[])
                            self.nc.register_instruction(car, overwrite=True)
                            out.append(car)
                        inst.sync_info = mybir.SyncInfo(
                            on_wait=[waits[-1]],
                            on_update=list(si.on_update) if si.on_update else [])
                    out.append(inst)
                insts[:] = out
            return super()._lower_ordered_insts(ordered)

        def _drain_and_barrier(self, tick_clock, wait_clock):
            clock = tick_clock.global_clock
            allocated = wait_clock.sems.allocated()
            for proc, tick in enumerate(clock):
                if tick > 0 and proc in allocated:
                    n = self.nc.sync.nop(nofuse=True, hint="tailwait")
                    n.wait_op(allocated[proc], tick, "sem-ge")
            self.nc.sync.drain()
            self.nc.all_engine_barrier()
            assert self.sems is not None
            popped = self.nc._tile_sem_poison_stack.pop()
            assert popped is self._sem_poison
            self.nc.clear_and_free_semaphores(list(self.sems.allocated().values()))
            self.nc.all_engine_barrier()

    return _TC(nc)


def _build_program(k_lists, variant="full"):
    """k_lists: per sub-tile s, list over rounds r of k_{r,s} (number of
    suffix blocks the round covers). k_lists[s][0] == BPS always."""
    import concourse.bass as bass
    import concourse.mybir as mybir

    do_gather = variant in ("full", "gather")
    do_conv = variant in ("full", "conv")

    f32 = mybir.dt.float32
    bf16 = mybir.dt.bfloat16
    AF = mybir.ActivationFunctionType
    ALU = mybir.AluOpType

    gdt = bf16 if GATHER_BF16 else f32
    nc = bass.Bass()
    g_d = {}
    for s in range(NSUB):
        for r, k in enumerate(k_lists[s]):
            g_d[(s, r)] = nc.dram_tensor(
                f"g{s}_{r}", [128, k * HW], gdt, kind="ExternalInput")
    fown_d = nc.dram_tensor("fown", [64, NB * HW], bf16, kind="ExternalInput")
    w1_d = nc.dram_tensor("w1bd", [128, 9 * 128], bf16, kind="ExternalInput")
    w1s_d = nc.dram_tensor("w1sbd", [64, 9 * 128], bf16, kind="ExternalInput")
    w2_d = nc.dram_tensor("w2bd", [128, 9 * 128], bf16, kind="ExternalInput")
    w3_d = nc.dram_tensor("w3bd", [128, 9 * 64], bf16, kind="ExternalInput")
    y_d = nc.dram_tensor("y", [64, NB * HW], f32, kind="ExternalOutput")

    def valid(ap_grid):
        # [p, GRID] tile AP -> [p, 32, 32] interior view of the 34x34 grid
        return ap_grid.rearrange("p (h w) -> p h w", w=WP)[:, 1:H + 1, 1:W + 1]

    def tap_view(ap_grid, k, dy, dx, h0):
        g3 = ap_grid.rearrange("p (h w) -> p h w", w=WP)
        return g3[0:k, h0 + dy:h0 + dy + 16, dx:dx + W]

    tc = _make_tile_context(nc)
    with tc:
        with (tc.tile_pool(name="cw", bufs=1) as cw,
              tc.tile_pool(name="x1pnp", bufs=3) as x1pnp,
              tc.tile_pool(name="x1sp", bufs=3) as x1sp,
              tc.tile_pool(name="fop", bufs=3) as fop,
              tc.tile_pool(name="x2p", bufs=3) as x2p,
              tc.tile_pool(name="x3p", bufs=3) as x3p,
              tc.tile_pool(name="r2p", bufs=3) as r2p,
              tc.tile_pool(name="r2cp", bufs=3) as r2cp,
              tc.tile_pool(name="osbp", bufs=3) as osbp,
              tc.tile_pool(name="psp", bufs=4, space="PSUM") as psp):
            w1_t = cw.tile([128, 9 * 128], bf16)
            nc.sync.dma_start(out=w1_t[:], in_=w1_d[:])
            w1s_t = cw.tile([64, 9 * 128], bf16)
            nc.sync.dma_start(out=w1s_t[:], in_=w1s_d[:])
            w2_t = cw.tile([128, 9 * 128], bf16)
            nc.sync.dma_start(out=w2_t[:], in_=w2_d[:])
            w3_t = cw.tile([128, 9 * 64], bf16)
            nc.sync.dma_start(out=w3_t[:], in_=w3_d[:])

            # pooled accumulator sub-tiles, f32
            A = [cw.tile([128, BPS * HW], f32, tag=f"A{s}", name=f"A{s}")
                 for s in range(NSUB)]
            if do_gather:
                # round 0: bulk bypass init; rounds >=1: per-block accum DMAs
                # (CCE accumulate breaks on >2-dim APs / >8KB runs)
                for s in range(NSUB):
                    for r, k in enumerate(k_lists[s]):
                        if r == 0:
                            nc.gpsimd.dma_start(out=A[s][:], in_=g_d[(s, 0)][:])
                            continue
                        for jj in range(k):
                            j = (BPS - k) + jj
                            nc.gpsimd.dma_start(
                                out=A[s][:, j * HW:(j + 1) * HW],
                                in_=g_d[(s, r)][:, jj * HW:(jj + 1) * HW],
                                accum_op=ALU.add)
            else:
                for s in range(NSUB):
                    nc.vector.memset(A[s][:], 0.0)

            memset_count = {}

            def fresh_grid(pool, name, parts=128):
                t = pool.tile([parts, GRID], bf16, tag=name)
                c = memset_count.get(name, 0)
                if c < 3:  # pool bufs
                    nc.vector.memset(t[:], 0.0)
                    memset_count[name] = c + 1
                return t

            def conv1(g):
                s, j = g // BPS, g % BPS
                x1 = fresh_grid(x1pnp, "x1pn")
                nc.vector.tensor_copy(
                    out=valid(x1[:]),
                    in_=A[s][:, j * HW:(j + 1) * HW].rearrange(
                        "p (h w) -> p h w", w=W))
                fo = fop.tile([64, HW], bf16, tag="fo")
                nc.sync.dma_start(out=fo[:], in_=fown_d[:, g * HW:(g + 1) * HW])
                x1s = fresh_grid(x1sp, "x1s", parts=64)
                nc.vector.tensor_copy(
                    out=valid(x1s[:]),
                    in_=fo[:].rearrange("p (h w) -> p h w", w=W))
                ps1 = psp.tile([128, HW], f32, tag="ps")
                ps1v = ps1[:].rearrange("p (h w) -> p h w", w=W)
                for h0 in (0, 16):
                    for t in range(9):
                        nc.tensor.matmul(
                            out=ps1v[:, h0:h0 + 16, :],
                            lhsT=w1_t[:, 128 * t:128 * t + 128],
                            rhs=tap_view(x1[:], 128, t // 3, t % 3, h0),
                            start=(t == 0), stop=False)
                    for t in range(9):
                        nc.tensor.matmul(
                            out=ps1v[:, h0:h0 + 16, :],
                            lhsT=w1s_t[:, 128 * t:128 * t + 128],
                            rhs=tap_view(x1s[:], 64, t // 3, t % 3, h0),
                            start=False, stop=(t == 8))
                r2a = r2p.tile([128, HW], bf16, tag="r2")
                nc.scalar.activation(out=r2a[:], in_=ps1[:], func=AF.Relu,
                                     scale=-0.9)
                x2 = fresh_grid(x2p, "x2")
                nc.vector.tensor_tensor(
                    out=valid(x2[:]),
                    in0=ps1[:].rearrange("p (h w) -> p h w", w=W),
                    in1=r2a[:].rearrange("p (h w) -> p h w", w=W),
                    op=ALU.add)
                return x2

            def conv2(x2):
                ps2 = psp.tile([128, HW], f32, tag="ps")
                ps2v = ps2[:].rearrange("p (h w) -> p h w", w=W)
                for h0 in (0, 16):
                    for t in range(9):
                        nc.tensor.matmul(
                            out=ps2v[:, h0:h0 + 16, :],
                            lhsT=w2_t[:, 128 * t:128 * t + 128],
                            rhs=tap_view(x2[:], 128, t // 3, t % 3, h0),
                            start=(t == 0), stop=(t == 8))
                r2b = r2p.tile([128, HW], bf16, tag="r2")
                nc.scalar.activation(out=r2b[:], in_=ps2[:], func=AF.Relu,
                                     scale=-0.9)
                x3 = fresh_grid(x3p, "x3")
                nc.vector.tensor_tensor(
                    out=valid(x3[:]),
                    in0=ps2[:].rearrange("p (h w) -> p h w", w=W),
                    in1=r2b[:].rearrange("p (h w) -> p h w", w=W),
                    op=ALU.add)
                return x3

            def conv3(x3, g):
                ps3 = psp.tile([128, HW], f32, tag="ps")
                ps3v = ps3[:].rearrange("p (h w) -> p h w", w=W)
                for h0 in (0, 16):
                    for t in range(9):
                        nc.tensor.matmul(
                            out=ps3v[0:64, h0:h0 + 16, :],
                            lhsT=w3_t[:, 64 * t:64 * t + 64],
                            rhs=tap_view(x3[:], 128, t // 3, t % 3, h0),
                            start=(t == 0), stop=(t == 8))
                r2c = r2cp.tile([64, HW], bf16, tag="r2c")
                nc.scalar.activation(out=r2c[:], in_=ps3[0:64, :], func=AF.Relu,
                                     scale=-0.9)
                osb = osbp.tile([64, HW], f32, tag="osb")
                nc.vector.tensor_tensor(out=osb[:], in0=ps3[0:64, :],
                                        in1=r2c[:], op=ALU.add)
                nc.sync.dma_start(out=y_d[:, g * HW:(g + 1) * HW], in_=osb[:])

            # software pipeline: pairs of bundles, layer-interleaved
            if do_conv:
                for g0 in range(0, NB, 2):
                    x2a = conv1(g0)
                    x2b = conv1(g0 + 1)
                    x3a = conv2(x2a)
                    x3b = conv2(x2b)
                    conv3(x3a, g0)
                    conv3(x3b, g0 + 1)
            else:
                # dump pooled accumulator (partitions 0:64) for inspection
                for g in range(NB):
                    s, j = g // BPS, g % BPS
                    nc.sync.dma_start(
                        out=y_d[:, g * HW:(g + 1) * HW],
                        in_=A[s][0:64, j * HW:(j + 1) * HW])
    return nc


def _host_prep(feats, edges, w1, b1, w2, b2, w3, b3):
    import ml_dtypes

    feats = np.ascontiguousarray(np.asarray(feats, dtype=np.float32))
    edges = np.asarray(edges)
    w1 = np.asarray(w1, dtype=np.float32)
    w2 = np.asarray(w2, dtype=np.float32)
    w3 = np.asarray(w3, dtype=np.float32)

    # per-(node, sign) contribution lists (bidirectional)
    contrib = [([], []) for _ in range(N)]
    for s, sg, d in edges.tolist():
        si = 0 if sg > 0 else 1
        contrib[d][si].append(s)
        contrib[s][si].append(d)

    # per-core slot ordering: ascending max-degree so low-degree bundles are
    # gathered first and consumed first by the conv pipeline
    slot2node = []
    for k in range(NCORES):
        nodes = list(range(NPC * k, NPC * (k + 1)))
        nodes.sort(key=lambda n: max(len(contrib[n][0]), len(contrib[n][1])))
        slot2node.append(nodes)

    # block max degree per (core, bundle)
    bmax = np.zeros((NCORES, NB), np.int64)
    for k in range(NCORES):
        for g in range(NB):
            m = 0
            for jm in range(4):
                n = slot2node[k][4 * g + jm]
                m = max(m, len(contrib[n][0]), len(contrib[n][1]))
            bmax[k, g] = m

    # per-sub-tile round coverage, uniform across cores
    k_lists = []
    for s in range(NSUB):
        blk = bmax[:, s * BPS:(s + 1) * BPS]  # [cores, BPS], ascending per core
        rmax = int(blk.max())
        ks = []
        for r in range(max(rmax, 1)):
            k = int((blk > r).sum(axis=1).max()) if r > 0 else BPS
            ks.append(max(k, 1) if r > 0 else BPS)
        k_lists.append(ks)

    featsN = feats.reshape(N, C, HW)
    tabN = np.concatenate([featsN, np.zeros((1, C, HW), np.float32)], axis=0)
    tabN_bf = tabN.astype(ml_dtypes.bfloat16)
    tabN_g = tabN_bf if GATHER_BF16 else tabN

    # weight tiles: block-diagonal lhsT layouts
    def bd_tile(wsel, ci_n, co_n):
        # wsel: [co, ci_n, 3, 3]; returns [4*ci_n(? partitions), 9*128-ish]
        t = np.zeros((4 * ci_n, 9 * 4 * co_n), np.float32)
        for jm in range(4):
            for tp in range(9):
                dy, dx = tp // 3, tp % 3
                t[ci_n * jm:ci_n * (jm + 1),
                  4 * co_n * tp + co_n * jm:4 * co_n * tp + co_n * (jm + 1)] = \
                    wsel[:, :, dy, dx].T
        return t.astype(ml_dtypes.bfloat16)

    w1bd = bd_tile(w1[:, C:3 * C], 2 * C, 2 * C)      # [128, 9*128] pos+neg
    w1sbd = bd_tile(w1[:, 0:C], C, 2 * C)             # [64, 9*128] self
    w2bd = bd_tile(w2, 2 * C, 2 * C)                  # [128, 9*128]
    w3bd = bd_tile(w3, 2 * C, C)                      # [128, 9*64]

    in_maps = []
    for k in range(NCORES):
        m = {"w1bd": w1bd, "w1sbd": w1sbd, "w2bd": w2bd, "w3bd": w3bd}
        nodes_k = np.array(slot2node[k]).reshape(NB, 4)  # [jM, jm]
        # fown: [64, NB*HW] partitions p=16*jm+c, free = jM*HW + px
        fo = tabN_bf[nodes_k]                  # [jM, jm, C, HW]
        m["fown"] = np.ascontiguousarray(
            fo.transpose(1, 2, 0, 3).reshape(64, NB * HW))
        # gather tables
        for s in range(NSUB):
            for r, kk in enumerate(k_lists[s]):
                # blocks covered: jM in [s*BPS + BPS-kk, (s+1)*BPS)
                srcs = np.full((8, kk), N, np.int64)  # default: zero row
                for jj in range(kk):
                    g = s * BPS + (BPS - kk) + jj
                    for jm in range(4):
                        n = slot2node[k][4 * g + jm]
                        for sg in range(2):
                            lst = contrib[n][sg]
                            if r < len(lst):
                                srcs[2 * jm + sg, jj] = lst[r]
                arr = tabN_g[srcs]             # [8, kk, C, HW]
                m[f"g{s}_{r}"] = np.ascontiguousarray(
                    arr.transpose(0, 2, 1, 3).reshape(128, kk * HW))
        in_maps.append(m)
    return in_maps, slot2node, tuple(tuple(ks) for ks in k_lists)


def kernel(feats, edges, w1, b1, w2, b2, w3, b3):
    from concourse.bass_utils import run_bass_kernel_spmd

    with_bias = bool(np.any(np.asarray(b1)) or np.any(np.asarray(b2))
                     or np.any(np.asarray(b3)))
    assert not with_bias, "nonzero conv biases not implemented"

    in_maps, slot2node, k_key = _host_prep(
        feats, edges, w1, b1, w2, b2, w3, b3)

    nc = _prog_cache.get(k_key)
    if nc is None:
        nc = _build_program([list(ks) for ks in k_key])
        _prog_cache[k_key] = nc

    import os
    trace = bool(os.environ.get("KERNEL_TRACE"))
    res = run_bass_kernel_spmd(nc, in_maps, core_ids=list(range(NCORES)),
                               trace=trace)
    if trace:
        global last_results
        last_results = res

    out = np.empty((N, C, H, W), np.float32)
    for k in range(NCORES):
        yk = res.results[k]["y"].reshape(4, C, NB, HW)  # [jm, c, jM, px]
        for g in range(NB):
            for jm in range(4):
                n = slot2node[k][4 * g + jm]
                out[n] = yk[jm, :, g, :].reshape(C, H, W)
    return out


# revision 19
# speedup vs baseline: 7.7479x; 1.2349x over previous
"""GNN message passing + 3x conv3x3 + leaky-relu, distributed over 8 trn2 NeuronCores.

Strategy v2 (node-sharded, 128 nodes/core):
- Pooling: host reorders neighbor features into per-round contiguous DRAM
  tables (one row per SBUF partition); the device accumulates them into a
  resident pooled mega-tile with a handful of large SWDGE DMAs using
  cce accumulate (bf16 source rows, f32 accumulation via dge-cast).
  Slots are degree-sorted so round r covers a contiguous suffix of node
  blocks -> zero indirection, ~64KB descriptors.
- Convs: 4 nodes packed per 128-partition bundle with BLOCK-DIAGONAL
  weights -> K=128 single matmuls instead of 4x (32x32) tile-position
  matmuls. 4x fewer PE instructions for the same math. 9 shifted-tap
  matmuls over a 34x34 zero-padded grid (no im2col), bf16 operands,
  fp32 PSUM accumulation.
- leaky_relu(x) = x + relu(-0.9 x): ScalarE activation + VectorE add.
- Bundles are consumed lowest-degree-first so conv overlaps the tail of
  the gather chain; pooled tensor is split into 4 sub-tiles so tile
  dependencies stay coarse but overlappable.
"""

import numpy as np

N, C, H, W = 1024, 16, 32, 32
NCORES = 8
NPC = N // NCORES            # nodes per core
NB = NPC // 4                # bundles (4 nodes) per core
NSUB = 8                     # pooled-accumulator sub-tiles
BPS = NB // NSUB             # bundles per sub-tile
HP = WP = H + 2
GRID = HP * WP
HW = H * W

GATHER_BF16 = True  # gather-table dtype; f32 accumulate either way

_prog_cache = {}


def _make_tile_context(nc):
    """TileContext whose lowering splits multi-sem waits onto nop carriers
    (this walrus build accepts at most one sync wait per instruction) and
    whose tail drain does the same."""
    import concourse.mybir as mybir
    import concourse.tile as tile

    class _TC(tile.TileContext):
        def _lower_ordered_insts(self, ordered):
            for bb_name, insts in ordered.items():
                out = []
                for inst in insts:
                    si = inst.sync_info
                    waits = list(si.on_wait) if si is not None and si.on_wait else []
                    if len(waits) > 1:
                        for w in waits[:-1]:
                            car = mybir.InstNoOp(
                                name=self.nc.get_next_instruction_name(),
                                ins=[], outs=[])
                            car.engine = inst.engine
                            car.sync_info = mybir.SyncInfo(on_wait=[w], on_update=[])
                            self.nc.register_instruction(car, overwrite=True)
                            out.append(car)
                        inst.sync_info = mybir.SyncInfo(
                            on_wait=[waits[-1]],
                            on_update=list(si.on_update) if si.on_update else [])
                    out.append(inst)
                insts[:] = out
            return super()._lower_ordered_insts(ordered)

        def _drain_and_barrier(self, tick_clock, wait_clock):
            clock = tick_clock.global_clock
            allocated = wait_clock.sems.allocated()
            for proc, tick in enumerate(clock):
                if tick > 0 and proc in allocated:
                    n = self.nc.sync.nop(nofuse=True, hint="tailwait")
                    n.wait_op(allocated[proc], tick, "sem-ge")
            self.nc.sync.drain()
            self.nc.all_engine_barrier()
            assert self.sems is not None
            popped = self.nc._tile_sem_poison_stack.pop()
            assert popped is self._sem_poison
            self.nc.clear_and_free_semaphores(list(self.sems.allocated().values()))
            self.nc.all_engine_barrier()

    return _TC(nc)


def _build_program(k_lists, variant="full"):
    """k_lists: per sub-tile s, list over rounds r of k_{r,s} (number of
    suffix blocks the round covers). k_lists[s][0] == BPS always."""
    import concourse.bass as bass
    import concourse.mybir as mybir

    do_gather = variant in ("full", "gather")
    do_conv = variant in ("full", "conv")

    f32 = mybir.dt.float32
    bf16 = mybir.dt.bfloat16
    AF = mybir.ActivationFunctionType
    ALU = mybir.AluOpType

    gdt = bf16 if GATHER_BF16 else f32
    nc = bass.Bass()
    g_d = {}
    for s in range(NSUB):
        for r, k in enumerate(k_lists[s]):
            g_d[(s, r)] = nc.dram_tensor(
                f"g{s}_{r}", [128, k * HW], gdt, kind="ExternalInput")
    fown_d = nc.dram_tensor("fown", [64, NB * HW], bf16, kind="ExternalInput")
    w1_d = nc.dram_tensor("w1bd", [128, 9 * 128], bf16, kind="ExternalInput")
    w1s_d = nc.dram_tensor("w1sbd", [64, 9 * 128], bf16, kind="ExternalInput")
    w2_d = nc.dram_tensor("w2bd", [128, 9 * 128], bf16, kind="ExternalInput")
    w3_d = nc.dram_tensor("w3bd", [128, 9 * 64], bf16, kind="ExternalInput")
    y_d = nc.dram_tensor("y", [64, NB * HW], f32, kind="ExternalOutput")

    def valid(ap_grid):
        # [p, GRID] tile AP -> [p, 32, 32] interior view of the 34x34 grid
        return ap_grid.rearrange("p (h w) -> p h w", w=WP)[:, 1:H + 1, 1:W + 1]

    def tap_view(ap_grid, k, dy, dx, h0):
        g3 = ap_grid.rearrange("p (h w) -> p h w", w=WP)
        return g3[0:k, h0 + dy:h0 + dy + 16, dx:dx + W]

    tc = _make_tile_context(nc)
    with tc:
        with (tc.tile_pool(name="cw", bufs=1) as cw,
              tc.tile_pool(name="x1pnp", bufs=3) as x1pnp,
              tc.tile_pool(name="x1sp", bufs=3) as x1sp,
              tc.tile_pool(name="fop", bufs=3) as fop,
              tc.tile_pool(name="x2p", bufs=3) as x2p,
              tc.tile_pool(name="x3p", bufs=3) as x3p,
              tc.tile_pool(name="r2p", bufs=3) as r2p,
              tc.tile_pool(name="r2cp", bufs=3) as r2cp,
              tc.tile_pool(name="osbp", bufs=3) as osbp,
              tc.tile_pool(name="psp", bufs=4, space="PSUM") as psp):
            w1_t = cw.tile([128, 9 * 128], bf16)
            nc.sync.dma_start(out=w1_t[:], in_=w1_d[:])
            w1s_t = cw.tile([64, 9 * 128], bf16)
            nc.sync.dma_start(out=w1s_t[:], in_=w1s_d[:])
            w2_t = cw.tile([128, 9 * 128], bf16)
            nc.sync.dma_start(out=w2_t[:], in_=w2_d[:])
            w3_t = cw.tile([128, 9 * 64], bf16)
            nc.sync.dma_start(out=w3_t[:], in_=w3_d[:])

            # pooled accumulator sub-tiles, f32
            A = [cw.tile([128, BPS * HW], f32, tag=f"A{s}", name=f"A{s}")
                 for s in range(NSUB)]
            if do_gather:
                # round 0: bulk bypass init; rounds >=1: per-block accum DMAs
                # (CCE accumulate breaks on >2-dim APs / >8KB runs)
                for s in range(NSUB):
                    for r, k in enumerate(k_lists[s]):
                        if r == 0:
                            nc.gpsimd.dma_start(out=A[s][:], in_=g_d[(s, 0)][:])
                            continue
                        for jj in range(k):
                            j = (BPS - k) + jj
                            nc.gpsimd.dma_start(
                                out=A[s][:, j * HW:(j + 1) * HW],
                                in_=g_d[(s, r)][:, jj * HW:(jj + 1) * HW],
                                accum_op=ALU.add)
            else:
                for s in range(NSUB):
                    nc.vector.memset(A[s][:], 0.0)

            memset_count = {}

            def fresh_grid(pool, name, parts=128):
                t = pool.tile([parts, GRID], bf16, tag=name)
                c = memset_count.get(name, 0)
                if c < 3:  # pool bufs
                    nc.vector.memset(t[:], 0.0)
                    memset_count[name] = c + 1
                return t

            def conv1(g):
                s, j = g // BPS, g % BPS
                x1 = fresh_grid(x1pnp, "x1pn")
                nc.vector.tensor_copy(
                    out=valid(x1[:]),
                    in_=A[s][:, j * HW:(j + 1) * HW].rearrange(
                        "p (h w) -> p h w", w=W))
                fo = fop.tile([64, HW], bf16, tag="fo")
                nc.sync.dma_start(out=fo[:], in_=fown_d[:, g * HW:(g + 1) * HW])
                x1s = fresh_grid(x1sp, "x1s", parts=64)
                nc.vector.tensor_copy(
                    out=valid(x1s[:]),
                    in_=fo[:].rearrange("p (h w) -> p h w", w=W))
                ps1 = psp.tile([128, HW], f32, tag="ps")
                ps1v = ps1[:].rearrange("p (h w) -> p h w", w=W)
                for t in range(9):
                    for h0 in (0, 16):
                        nc.tensor.matmul(
                            out=ps1v[:, h0:h0 + 16, :],
                            lhsT=w1_t[:, 128 * t:128 * t + 128],
                            rhs=tap_view(x1[:], 128, t // 3, t % 3, h0),
                            start=(t == 0), stop=False)
                for t in range(9):
                    for h0 in (0, 16):
                        nc.tensor.matmul(
                            out=ps1v[:, h0:h0 + 16, :],
                            lhsT=w1s_t[:, 128 * t:128 * t + 128],
                            rhs=tap_view(x1s[:], 64, t // 3, t % 3, h0),
                            start=False, stop=(t == 8))
                r2a = r2p.tile([128, HW], bf16, tag="r2")
                nc.scalar.activation(out=r2a[:], in_=ps1[:], func=AF.Relu,
                                     scale=-0.9)
                x2 = fresh_grid(x2p, "x2")
                nc.vector.tensor_tensor(
                    out=valid(x2[:]),
                    in0=ps1[:].rearrange("p (h w) -> p h w", w=W),
                    in1=r2a[:].rearrange("p (h w) -> p h w", w=W),
                    op=ALU.add)
                return x2

            def conv2(x2):
                ps2 = psp.tile([128, HW], f32, tag="ps")
                ps2v = ps2[:].rearrange("p (h w) -> p h w", w=W)
                for t in range(9):
                    for h0 in (0, 16):
                        nc.tensor.matmul(
                            out=ps2v[:, h0:h0 + 16, :],
                            lhsT=w2_t[:, 128 * t:128 * t + 128],
                            rhs=tap_view(x2[:], 128, t // 3, t % 3, h0),
                            start=(t == 0), stop=(t == 8))
                r2b = r2p.tile([128, HW], bf16, tag="r2")
                nc.scalar.activation(out=r2b[:], in_=ps2[:], func=AF.Relu,
                                     scale=-0.9)
                x3 = fresh_grid(x3p, "x3")
                nc.vector.tensor_tensor(
                    out=valid(x3[:]),
                    in0=ps2[:].rearrange("p (h w) -> p h w", w=W),
                    in1=r2b[:].rearrange("p (h w) -> p h w", w=W),
                    op=ALU.add)
                return x3

            def conv3(x3, g):
                ps3 = psp.tile([128, HW], f32, tag="ps")
                ps3v = ps3[:].rearrange("p (h w) -> p h w", w=W)
                for t in range(9):
                    for h0 in (0, 16):
                        nc.tensor.matmul(
                            out=ps3v[0:64, h0:h0 + 16, :],
                            lhsT=w3_t[:, 64 * t:64 * t + 64],
                            rhs=tap_view(x3[:], 128, t // 3, t % 3, h0),
                            start=(t == 0), stop=(t == 8))
                r2c = r2cp.tile([64, HW], bf16, tag="r2c")
                nc.scalar.activation(out=r2c[:], in_=ps3[0:64, :], func=AF.Relu,
                                     scale=-0.9)
                osb = osbp.tile([64, HW], f32, tag="osb")
                nc.vector.tensor_tensor(out=osb[:], in0=ps3[0:64, :],
                                        in1=r2c[:], op=ALU.add)
                nc.sync.dma_start(out=y_d[:, g * HW:(g + 1) * HW], in_=osb[:])

            # software pipeline: pairs of bundles, layer-interleaved
            if do_conv:
                for g0 in range(0, NB, 2):
                    x2a = conv1(g0)
                    x2b = conv1(g0 + 1)
                    x3a = conv2(x2a)
                    x3b = conv2(x2b)
                    conv3(x3a, g0)
                    conv3(x3b, g0 + 1)
            else:
                # dump pooled accumulator (partitions 0:64) for inspection
                for g in range(NB):
                    s, j = g // BPS, g % BPS
                    nc.sync.dma_start(
                        out=y_d[:, g * HW:(g + 1) * HW],
                        in_=A[s][0:64, j * HW:(j + 1) * HW])
    return nc


def _host_prep(feats, edges, w1, b1, w2, b2, w3, b3):
    import ml_dtypes

    feats = np.ascontiguousarray(np.asarray(feats, dtype=np.float32))
    edges = np.asarray(edges)
    w1 = np.asarray(w1, dtype=np.float32)
    w2 = np.asarray(w2, dtype=np.float32)
    w3 = np.asarray(w3, dtype=np.float32)

    # per-(node, sign) contribution lists (bidirectional)
    contrib = [([], []) for _ in range(N)]
    for s, sg, d in edges.tolist():
        si = 0 if sg > 0 else 1
        contrib[d][si].append(s)
        contrib[s][si].append(d)

    # per-core slot ordering: ascending max-degree so low-degree bundles are
    # gathered first and consumed first by the conv pipeline
    slot2node = []
    for k in range(NCORES):
        nodes = list(range(NPC * k, NPC * (k + 1)))
        nodes.sort(key=lambda n: max(len(contrib[n][0]), len(contrib[n][1])))
        slot2node.append(nodes)

    # block max degree per (core, bundle)
    bmax = np.zeros((NCORES, NB), np.int64)
    for k in range(NCORES):
        for g in range(NB):
            m = 0
            for jm in range(4):
                n = slot2node[k][4 * g + jm]
                m = max(m, len(contrib[n][0]), len(contrib[n][1]))
            bmax[k, g] = m

    # per-sub-tile round coverage, uniform across cores
    k_lists = []
    for s in range(NSUB):
        blk = bmax[:, s * BPS:(s + 1) * BPS]  # [cores, BPS], ascending per core
        rmax = int(blk.max())
        ks = []
        for r in range(max(rmax, 1)):
            k = int((blk > r).sum(axis=1).max()) if r > 0 else BPS
            ks.append(max(k, 1) if r > 0 else BPS)
        k_lists.append(ks)

    featsN = feats.reshape(N, C, HW)
    tabN = np.concatenate([featsN, np.zeros((1, C, HW), np.float32)], axis=0)
    tabN_bf = tabN.astype(ml_dtypes.bfloat16)
    tabN_g = tabN_bf if GATHER_BF16 else tabN

    # weight tiles: block-diagonal lhsT layouts
    def bd_tile(wsel, ci_n, co_n):
        # wsel: [co, ci_n, 3, 3]; returns [4*ci_n(? partitions), 9*128-ish]
        t = np.zeros((4 * ci_n, 9 * 4 * co_n), np.float32)
        for jm in range(4):
            for tp in range(9):
                dy, dx = tp // 3, tp % 3
                t[ci_n * jm:ci_n * (jm + 1),
                  4 * co_n * tp + co_n * jm:4 * co_n * tp + co_n * (jm + 1)] = \
                    wsel[:, :, dy, dx].T
        return t.astype(ml_dtypes.bfloat16)

    w1bd = bd_tile(w1[:, C:3 * C], 2 * C, 2 * C)      # [128, 9*128] pos+neg
    w1sbd = bd_tile(w1[:, 0:C], C, 2 * C)             # [64, 9*128] self
    w2bd = bd_tile(w2, 2 * C, 2 * C)                  # [128, 9*128]
    w3bd = bd_tile(w3, 2 * C, C)                      # [128, 9*64]

    in_maps = []
    for k in range(NCORES):
        m = {"w1bd": w1bd, "w1sbd": w1sbd, "w2bd": w2bd, "w3bd": w3bd}
        nodes_k = np.array(slot2node[k]).reshape(NB, 4)  # [jM, jm]
        # fown: [64, NB*HW] partitions p=16*jm+c, free = jM*HW + px
        fo = tabN_bf[nodes_k]                  # [jM, jm, C, HW]
        m["fown"] = np.ascontiguousarray(
            fo.transpose(1, 2, 0, 3).reshape(64, NB * HW))
        # gather tables
        for s in range(NSUB):
            for r, kk in enumerate(k_lists[s]):
                # blocks covered: jM in [s*BPS + BPS-kk, (s+1)*BPS)
                srcs = np.full((8, kk), N, np.int64)  # default: zero row
                for jj in range(kk):
                    g = s * BPS + (BPS - kk) + jj
                    for jm in range(4):
                        n = slot2node[k][4 * g + jm]
                        for sg in range(2):
                            lst = contrib[n][sg]
                            if r < len(lst):
                                srcs[2 * jm + sg, jj] = lst[r]
                arr = tabN_g[srcs]             # [8, kk, C, HW]
                m[f"g{s}_{r}"] = np.ascontiguousarray(
                    arr.transpose(0, 2, 1, 3).reshape(128, kk * HW))
        in_maps.append(m)
    return in_maps, slot2node, tuple(tuple(ks) for ks in k_lists)


def kernel(feats, edges, w1, b1, w2, b2, w3, b3):
    from concourse.bass_utils import run_bass_kernel_spmd

    with_bias = bool(np.any(np.asarray(b1)) or np.any(np.asarray(b2))
                     or np.any(np.asarray(b3)))
    assert not with_bias, "nonzero conv biases not implemented"

    in_maps, slot2node, k_key = _host_prep(
        feats, edges, w1, b1, w2, b2, w3, b3)

    nc = _prog_cache.get(k_key)
    if nc is None:
        nc = _build_program([list(ks) for ks in k_key])
        _prog_cache[k_key] = nc

    import os
    trace = bool(os.environ.get("KERNEL_TRACE"))
    res = run_bass_kernel_spmd(nc, in_maps, core_ids=list(range(NCORES)),
                               trace=trace)
    if trace:
        global last_results
        last_results = res

    out = np.empty((N, C, H, W), np.float32)
    for k in range(NCORES):
        yk = res.results[k]["y"].reshape(4, C, NB, HW)  # [jm, c, jM, px]
        for g in range(NB):
            for jm in range(4):
                n = slot2node[k][4 * g + jm]
                out[n] = yk[jm, :, g, :].reshape(C, H, W)
    return out


# revision 25
# speedup vs baseline: 8.1883x; 1.0568x over previous
"""GNN message passing + 3x conv3x3 + leaky-relu, distributed over 8 trn2 NeuronCores.

Strategy v2 (node-sharded, 128 nodes/core):
- Pooling: host reorders neighbor features into per-round contiguous DRAM
  tables (one row per SBUF partition); the device accumulates them into a
  resident pooled mega-tile with a handful of large SWDGE DMAs using
  cce accumulate (bf16 source rows, f32 accumulation via dge-cast).
  Slots are degree-sorted so round r covers a contiguous suffix of node
  blocks -> zero indirection, ~64KB descriptors.
- Convs: 4 nodes packed per 128-partition bundle with BLOCK-DIAGONAL
  weights -> K=128 single matmuls instead of 4x (32x32) tile-position
  matmuls. 4x fewer PE instructions for the same math. 9 shifted-tap
  matmuls over a 34x34 zero-padded grid (no im2col), bf16 operands,
  fp32 PSUM accumulation.
- leaky_relu(x) = x + relu(-0.9 x): ScalarE activation + VectorE add.
- Bundles are consumed lowest-degree-first so conv overlaps the tail of
  the gather chain; pooled tensor is split into 4 sub-tiles so tile
  dependencies stay coarse but overlappable.
"""

import numpy as np

N, C, H, W = 1024, 16, 32, 32
NCORES = 8
NPC = N // NCORES            # nodes per core
NB = NPC // 4                # bundles (4 nodes) per core
NSUB = 8                     # pooled-accumulator sub-tiles
BPS = NB // NSUB             # bundles per sub-tile
HP = WP = H + 2
GRID = HP * WP
HW = H * W

GATHER_BF16 = True  # gather-table dtype; f32 accumulate either way

_prog_cache = {}


def _make_tile_context(nc):
    """TileContext whose lowering splits multi-sem waits onto nop carriers
    (this walrus build accepts at most one sync wait per instruction) and
    whose tail drain does the same."""
    import concourse.mybir as mybir
    import concourse.tile as tile

    class _TC(tile.TileContext):
        def _lower_ordered_insts(self, ordered):
            for bb_name, insts in ordered.items():
                out = []
                for inst in insts:
                    si = inst.sync_info
                    waits = list(si.on_wait) if si is not None and si.on_wait else []
                    if len(waits) > 1:
                        for w in waits[:-1]:
                            car = mybir.InstNoOp(
                                name=self.nc.get_next_instruction_name(),
                                ins=[], outs=[])
                            car.engine = inst.engine
                            car.sync_info = mybir.SyncInfo(on_wait=[w], on_update=[])
                            self.nc.register_instruction(car, overwrite=True)
                            out.append(car)
                        inst.sync_info = mybir.SyncInfo(
                            on_wait=[waits[-1]],
                            on_update=list(si.on_update) if si.on_update else [])
                    out.append(inst)
                insts[:] = out
            return super()._lower_ordered_insts(ordered)

        def _drain_and_barrier(self, tick_clock, wait_clock):
            clock = tick_clock.global_clock
            allocated = wait_clock.sems.allocated()
            for proc, tick in enumerate(clock):
                if tick > 0 and proc in allocated:
                    n = self.nc.sync.nop(nofuse=True, hint="tailwait")
                    n.wait_op(allocated[proc], tick, "sem-ge")
            self.nc.sync.drain()
            self.nc.all_engine_barrier()
            assert self.sems is not None
            popped = self.nc._tile_sem_poison_stack.pop()
            assert popped is self._sem_poison
            self.nc.clear_and_free_semaphores(list(self.sems.allocated().values()))
            self.nc.all_engine_barrier()

    return _TC(nc)


def _build_program(k_lists, variant="full"):
    """k_lists: per sub-tile s, list over rounds r of k_{r,s} (number of
    suffix blocks the round covers). k_lists[s][0] == BPS always."""
    import concourse.bass as bass
    import concourse.mybir as mybir

    do_gather = variant in ("full", "gather")
    do_conv = variant in ("full", "conv")

    f32 = mybir.dt.float32
    bf16 = mybir.dt.bfloat16
    AF = mybir.ActivationFunctionType
    ALU = mybir.AluOpType

    gdt = bf16 if GATHER_BF16 else f32
    nc = bass.Bass()
    g_d = {}
    for s in range(NSUB):
        for r, k in enumerate(k_lists[s]):
            g_d[(s, r)] = nc.dram_tensor(
                f"g{s}_{r}", [128, k * HW], gdt, kind="ExternalInput")
    fown_d = nc.dram_tensor("fown", [64, NB * HW], bf16, kind="ExternalInput")
    w1_d = nc.dram_tensor("w1bd", [128, 9 * 128], bf16, kind="ExternalInput")
    w1sp_d = nc.dram_tensor("w1sp", [128, 3 * 128], bf16, kind="ExternalInput")
    w1sq_d = nc.dram_tensor("w1sq", [128, 3 * 128], bf16, kind="ExternalInput")
    w2_d = nc.dram_tensor("w2bd", [128, 9 * 128], bf16, kind="ExternalInput")
    w3_d = nc.dram_tensor("w3bd", [128, 9 * 64], bf16, kind="ExternalInput")
    y_d = nc.dram_tensor("y", [64, NB * HW], f32, kind="ExternalOutput")

    def valid(ap_grid):
        # [p, GRID] tile AP -> [p, 32, 32] interior view of the 34x34 grid
        return ap_grid.rearrange("p (h w) -> p h w", w=WP)[:, 1:H + 1, 1:W + 1]

    def tap_view(ap_grid, k, dy, dx, h0):
        g3 = ap_grid.rearrange("p (h w) -> p h w", w=WP)
        return g3[0:k, h0 + dy:h0 + dy + 16, dx:dx + W]

    tc = _make_tile_context(nc)
    with tc:
        with (tc.tile_pool(name="cw", bufs=1) as cw,
              tc.tile_pool(name="x1pnp", bufs=3) as x1pnp,
              tc.tile_pool(name="x1sp", bufs=3) as x1sp,
              tc.tile_pool(name="fop", bufs=3) as fop,
              tc.tile_pool(name="x2p", bufs=3) as x2p,
              tc.tile_pool(name="x3p", bufs=3) as x3p,
              tc.tile_pool(name="r2p", bufs=3) as r2p,
              tc.tile_pool(name="r2cp", bufs=3) as r2cp,
              tc.tile_pool(name="osbp", bufs=3) as osbp,
              tc.tile_pool(name="psp", bufs=4, space="PSUM") as psp):
            w1_t = cw.tile([128, 9 * 128], bf16)
            nc.sync.dma_start(out=w1_t[:], in_=w1_d[:])
            w1sp_t = cw.tile([128, 3 * 128], bf16)
            nc.sync.dma_start(out=w1sp_t[:], in_=w1sp_d[:])
            w1sq_t = cw.tile([128, 3 * 128], bf16)
            nc.sync.dma_start(out=w1sq_t[:], in_=w1sq_d[:])
            w2_t = cw.tile([128, 9 * 128], bf16)
            nc.sync.dma_start(out=w2_t[:], in_=w2_d[:])
            w3_t = cw.tile([128, 9 * 64], bf16)
            nc.sync.dma_start(out=w3_t[:], in_=w3_d[:])

            # pooled accumulator sub-tiles, f32
            A = [cw.tile([128, BPS * HW], f32, tag=f"A{s}", name=f"A{s}")
                 for s in range(NSUB)]
            if do_gather:
                # round 0: bulk bypass init; rounds >=1: per-block accum DMAs
                # (CCE accumulate breaks on >2-dim APs / >8KB runs)
                for s in range(NSUB):
                    for r, k in enumerate(k_lists[s]):
                        if r == 0:
                            nc.gpsimd.dma_start(out=A[s][:], in_=g_d[(s, 0)][:])
                            continue
                        for jj in range(k):
                            j = (BPS - k) + jj
                            nc.gpsimd.dma_start(
                                out=A[s][:, j * HW:(j + 1) * HW],
                                in_=g_d[(s, r)][:, jj * HW:(jj + 1) * HW],
                                accum_op=ALU.add)
            else:
                for s in range(NSUB):
                    nc.vector.memset(A[s][:], 0.0)

            memset_count = {}

            def fresh_grid(pool, name, parts=128):
                t = pool.tile([parts, GRID], bf16, tag=name)
                c = memset_count.get(name, 0)
                if c < 3:  # pool bufs
                    nc.vector.memset(t[:], 0.0)
                    memset_count[name] = c + 1
                return t

            def conv1(g):
                s, j = g // BPS, g % BPS
                x1 = fresh_grid(x1pnp, "x1pn")
                nc.vector.tensor_copy(
                    out=valid(x1[:]),
                    in_=A[s][:, j * HW:(j + 1) * HW].rearrange(
                        "p (h w) -> p h w", w=W))
                fo = fop.tile([64, HW], bf16, tag="fo")
                nc.sync.dma_start(out=fo[:], in_=fown_d[:, g * HW:(g + 1) * HW])
                # self grid + column-shifted replica in partitions 64..127:
                # x1s[64+p, h, w] == x1s[p, h, w+1]
                x1s = fresh_grid(x1sp, "x1s")
                fov = fo[:].rearrange("p (h w) -> p h w", w=W)
                nc.vector.tensor_copy(out=valid(x1s[0:64]), in_=fov)
                x1sg = x1s[:].rearrange("p (h w) -> p h w", w=WP)
                nc.vector.tensor_copy(out=x1sg[64:128, 1:H + 1, 0:W], in_=fov)
                ps1 = psp.tile([128, HW], f32, tag="ps")
                ps1v = ps1[:].rearrange("p (h w) -> p h w", w=W)
                for t in range(9):
                    for h0 in (0, 16):
                        nc.tensor.matmul(
                            out=ps1v[:, h0:h0 + 16, :],
                            lhsT=w1_t[:, 128 * t:128 * t + 128],
                            rhs=tap_view(x1[:], 128, t // 3, t % 3, h0),
                            start=(t == 0), stop=False)
                x1sv = x1s[:].rearrange("p (h w) -> p h w", w=WP)
                for dy in range(3):
                    for h0 in (0, 16):
                        # taps (dy,0)+(dy,1) in one K=128 matmul via replica
                        nc.tensor.matmul(
                            out=ps1v[:, h0:h0 + 16, :],
                            lhsT=w1sp_t[:, 128 * dy:128 * dy + 128],
                            rhs=x1sv[0:128, h0 + dy:h0 + dy + 16, 0:W],
                            start=False, stop=False)
                    for h0 in (0, 16):
                        # tap (dy,2): replica shifted once more
                        nc.tensor.matmul(
                            out=ps1v[:, h0:h0 + 16, :],
                            lhsT=w1sq_t[64:128, 128 * dy:128 * dy + 128],
                            rhs=x1sv[64:128, h0 + dy:h0 + dy + 16, 1:W + 1],
                            start=False, stop=(dy == 2))
                r2a = r2p.tile([128, HW], bf16, tag="r2")
                nc.scalar.activation(out=r2a[:], in_=ps1[:], func=AF.Relu,
                                     scale=-0.9)
                x2 = fresh_grid(x2p, "x2")
                nc.vector.tensor_tensor(
                    out=valid(x2[:]),
                    in0=ps1[:].rearrange("p (h w) -> p h w", w=W),
                    in1=r2a[:].rearrange("p (h w) -> p h w", w=W),
                    op=ALU.add)
                return x2

            def conv2(x2):
                ps2 = psp.tile([128, HW], f32, tag="ps")
                ps2v = ps2[:].rearrange("p (h w) -> p h w", w=W)
                for t in range(9):
                    for h0 in (0, 16):
                        nc.tensor.matmul(
                            out=ps2v[:, h0:h0 + 16, :],
                            lhsT=w2_t[:, 128 * t:128 * t + 128],
                            rhs=tap_view(x2[:], 128, t // 3, t % 3, h0),
                            start=(t == 0), stop=(t == 8))
                r2b = r2p.tile([128, HW], bf16, tag="r2")
                nc.scalar.activation(out=r2b[:], in_=ps2[:], func=AF.Relu,
                                     scale=-0.9)
                x3 = fresh_grid(x3p, "x3")
                nc.vector.tensor_tensor(
                    out=valid(x3[:]),
                    in0=ps2[:].rearrange("p (h w) -> p h w", w=W),
                    in1=r2b[:].rearrange("p (h w) -> p h w", w=W),
                    op=ALU.add)
                return x3

            def conv3(x3, g):
                ps3 = psp.tile([128, HW], f32, tag="ps")
                ps3v = ps3[:].rearrange("p (h w) -> p h w", w=W)
                for t in range(9):
                    for h0 in (0, 16):
                        nc.tensor.matmul(
                            out=ps3v[0:64, h0:h0 + 16, :],
                            lhsT=w3_t[:, 64 * t:64 * t + 64],
                            rhs=tap_view(x3[:], 128, t // 3, t % 3, h0),
                            start=(t == 0), stop=(t == 8))
                r2c = r2cp.tile([64, HW], bf16, tag="r2c")
                nc.scalar.activation(out=r2c[:], in_=ps3[0:64, :], func=AF.Relu,
                                     scale=-0.9)
                osb = osbp.tile([64, HW], f32, tag="osb")
                nc.vector.tensor_tensor(out=osb[:], in0=ps3[0:64, :],
                                        in1=r2c[:], op=ALU.add)
                nc.sync.dma_start(out=y_d[:, g * HW:(g + 1) * HW], in_=osb[:])

            # software pipeline: pairs of bundles, layer-interleaved
            if do_conv:
                for g0 in range(0, NB, 2):
                    x2a = conv1(g0)
                    x2b = conv1(g0 + 1)
                    x3a = conv2(x2a)
                    x3b = conv2(x2b)
                    conv3(x3a, g0)
                    conv3(x3b, g0 + 1)
            else:
                # dump pooled accumulator (partitions 0:64) for inspection
                for g in range(NB):
                    s, j = g // BPS, g % BPS
                    nc.sync.dma_start(
                        out=y_d[:, g * HW:(g + 1) * HW],
                        in_=A[s][0:64, j * HW:(j + 1) * HW])
    return nc


def _host_prep(feats, edges, w1, b1, w2, b2, w3, b3):
    import ml_dtypes

    feats = np.ascontiguousarray(np.asarray(feats, dtype=np.float32))
    edges = np.asarray(edges)
    w1 = np.asarray(w1, dtype=np.float32)
    w2 = np.asarray(w2, dtype=np.float32)
    w3 = np.asarray(w3, dtype=np.float32)

    # per-(node, sign) contribution lists (bidirectional)
    contrib = [([], []) for _ in range(N)]
    for s, sg, d in edges.tolist():
        si = 0 if sg > 0 else 1
        contrib[d][si].append(s)
        contrib[s][si].append(d)

    # per-core slot ordering: ascending max-degree so low-degree bundles are
    # gathered first and consumed first by the conv pipeline
    slot2node = []
    for k in range(NCORES):
        nodes = list(range(NPC * k, NPC * (k + 1)))
        nodes.sort(key=lambda n: max(len(contrib[n][0]), len(contrib[n][1])))
        slot2node.append(nodes)

    # block max degree per (core, bundle)
    bmax = np.zeros((NCORES, NB), np.int64)
    for k in range(NCORES):
        for g in range(NB):
            m = 0
            for jm in range(4):
                n = slot2node[k][4 * g + jm]
                m = max(m, len(contrib[n][0]), len(contrib[n][1]))
            bmax[k, g] = m

    # per-sub-tile round coverage, uniform across cores
    k_lists = []
    for s in range(NSUB):
        blk = bmax[:, s * BPS:(s + 1) * BPS]  # [cores, BPS], ascending per core
        rmax = int(blk.max())
        ks = []
        for r in range(max(rmax, 1)):
            k = int((blk > r).sum(axis=1).max()) if r > 0 else BPS
            ks.append(max(k, 1) if r > 0 else BPS)
        k_lists.append(ks)

    featsN = feats.reshape(N, C, HW)
    tabN = np.concatenate([featsN, np.zeros((1, C, HW), np.float32)], axis=0)
    tabN_bf = tabN.astype(ml_dtypes.bfloat16)
    tabN_g = tabN_bf if GATHER_BF16 else tabN

    # weight tiles: block-diagonal lhsT layouts
    def bd_tile(wsel, ci_n, co_n):
        # wsel: [co, ci_n, 3, 3]; returns [4*ci_n(? partitions), 9*128-ish]
        t = np.zeros((4 * ci_n, 9 * 4 * co_n), np.float32)
        for jm in range(4):
            for tp in range(9):
                dy, dx = tp // 3, tp % 3
                t[ci_n * jm:ci_n * (jm + 1),
                  4 * co_n * tp + co_n * jm:4 * co_n * tp + co_n * (jm + 1)] = \
                    wsel[:, :, dy, dx].T
        return t.astype(ml_dtypes.bfloat16)

    w1bd = bd_tile(w1[:, C:3 * C], 2 * C, 2 * C)      # [128, 9*128] pos+neg
    w2bd = bd_tile(w2, 2 * C, 2 * C)                  # [128, 9*128]
    w3bd = bd_tile(w3, 2 * C, C)                      # [128, 9*64]
    # conv1-self paired-tap weights: rows 0:64 self ch for dx=0 (w1sp) /
    # zero (w1sq); rows 64:128 replica ch for dx=1 (w1sp) / dx=2 (w1sq)
    w1sp = np.zeros((128, 3 * 128), np.float32)
    w1sq = np.zeros((128, 3 * 128), np.float32)
    for jm in range(4):
        for dy in range(3):
            blk = slice(128 * dy + 32 * jm, 128 * dy + 32 * jm + 32)
            w1sp[16 * jm:16 * jm + 16, blk] = w1[:, 0:C, dy, 0].T
            w1sp[64 + 16 * jm:64 + 16 * jm + 16, blk] = w1[:, 0:C, dy, 1].T
            w1sq[64 + 16 * jm:64 + 16 * jm + 16, blk] = w1[:, 0:C, dy, 2].T
    w1sp = w1sp.astype(ml_dtypes.bfloat16)
    w1sq = w1sq.astype(ml_dtypes.bfloat16)

    in_maps = []
    for k in range(NCORES):
        m = {"w1bd": w1bd, "w1sp": w1sp, "w1sq": w1sq,
             "w2bd": w2bd, "w3bd": w3bd}
        nodes_k = np.array(slot2node[k]).reshape(NB, 4)  # [jM, jm]
        # fown: [64, NB*HW] partitions p=16*jm+c, free = jM*HW + px
        fo = tabN_bf[nodes_k]                  # [jM, jm, C, HW]
        m["fown"] = np.ascontiguousarray(
            fo.transpose(1, 2, 0, 3).reshape(64, NB * HW))
        # gather tables
        for s in range(NSUB):
            for r, kk in enumerate(k_lists[s]):
                # blocks covered: jM in [s*BPS + BPS-kk, (s+1)*BPS)
                srcs = np.full((8, kk), N, np.int64)  # default: zero row
                for jj in range(kk):
                    g = s * BPS + (BPS - kk) + jj
                    for jm in range(4):
                        n = slot2node[k][4 * g + jm]
                        for sg in range(2):
                            lst = contrib[n][sg]
                            if r < len(lst):
                                srcs[2 * jm + sg, jj] = lst[r]
                arr = tabN_g[srcs]             # [8, kk, C, HW]
                m[f"g{s}_{r}"] = np.ascontiguousarray(
                    arr.transpose(0, 2, 1, 3).reshape(128, kk * HW))
        in_maps.append(m)
    return in_maps, slot2node, tuple(tuple(ks) for ks in k_lists)


def kernel(feats, edges, w1, b1, w2, b2, w3, b3):
    from concourse.bass_utils import run_bass_kernel_spmd

    with_bias = bool(np.any(np.asarray(b1)) or np.any(np.asarray(b2))
                     or np.any(np.asarray(b3)))
    assert not with_bias, "nonzero conv biases not implemented"

    in_maps, slot2node, k_key = _host_prep(
        feats, edges, w1, b1, w2, b2, w3, b3)

    nc = _prog_cache.get(k_key)
    if nc is None:
        nc = _build_program([list(ks) for ks in k_key])
        _prog_cache[k_key] = nc

    import os
    trace = bool(os.environ.get("KERNEL_TRACE"))
    res = run_bass_kernel_spmd(nc, in_maps, core_ids=list(range(NCORES)),
                               trace=trace)
    if trace:
        global last_results
        last_results = res

    out = np.empty((N, C, H, W), np.float32)
    for k in range(NCORES):
        yk = res.results[k]["y"].reshape(4, C, NB, HW)  # [jm, c, jM, px]
        for g in range(NB):
            for jm in range(4):
                n = slot2node[k][4 * g + jm]
                out[n] = yk[jm, :, g, :].reshape(C, H, W)
    return out


# revision 34
# speedup vs baseline: 8.3075x; 1.0146x over previous
"""GNN message passing + 3x conv3x3 + leaky-relu, distributed over 8 trn2 NeuronCores.

Strategy v2 (node-sharded, 128 nodes/core):
- Pooling: host reorders neighbor features into per-round contiguous DRAM
  tables (one row per SBUF partition); the device accumulates them into a
  resident pooled mega-tile with a handful of large SWDGE DMAs using
  cce accumulate (bf16 source rows, f32 accumulation via dge-cast).
  Slots are degree-sorted so round r covers a contiguous suffix of node
  blocks -> zero indirection, ~64KB descriptors.
- Convs: 4 nodes packed per 128-partition bundle with BLOCK-DIAGONAL
  weights -> K=128 single matmuls instead of 4x (32x32) tile-position
  matmuls. 4x fewer PE instructions for the same math. 9 shifted-tap
  matmuls over a 34x34 zero-padded grid (no im2col), bf16 operands,
  fp32 PSUM accumulation.
- leaky_relu(x) = x + relu(-0.9 x): ScalarE activation + VectorE add.
- Bundles are consumed lowest-degree-first so conv overlaps the tail of
  the gather chain; pooled tensor is split into 4 sub-tiles so tile
  dependencies stay coarse but overlappable.
"""

import numpy as np

N, C, H, W = 1024, 16, 32, 32
NCORES = 8
NPC = N // NCORES            # nodes per core
NB = NPC // 4                # bundles (4 nodes) per core
SUBS = [1, 1, 2, 4, 8, 8, 8]  # pooled-accumulator sub-tile sizes (bundles)
SUB0 = [sum(SUBS[:i]) for i in range(len(SUBS))]  # first bundle of each sub
NSUB = len(SUBS)
assert sum(SUBS) == NB
HP = WP = H + 2
GRID = HP * WP
HW = H * W

GATHER_BF16 = True  # gather-table dtype; f32 accumulate either way

_prog_cache = {}


def _make_tile_context(nc):
    """TileContext whose lowering splits multi-sem waits onto nop carriers
    (this walrus build accepts at most one sync wait per instruction) and
    whose tail drain does the same."""
    import concourse.mybir as mybir
    import concourse.tile as tile

    class _TC(tile.TileContext):
        def _lower_ordered_insts(self, ordered):
            for bb_name, insts in ordered.items():
                out = []
                for inst in insts:
                    si = inst.sync_info
                    waits = list(si.on_wait) if si is not None and si.on_wait else []
                    if len(waits) > 1:
                        for w in waits[:-1]:
                            car = mybir.InstNoOp(
                                name=self.nc.get_next_instruction_name(),
                                ins=[], outs=[])
                            car.engine = inst.engine
                            car.sync_info = mybir.SyncInfo(on_wait=[w], on_update=[])
                            self.nc.register_instruction(car, overwrite=True)
                            out.append(car)
                        inst.sync_info = mybir.SyncInfo(
                            on_wait=[waits[-1]],
                            on_update=list(si.on_update) if si.on_update else [])
                    out.append(inst)
                insts[:] = out
            return super()._lower_ordered_insts(ordered)

        def _drain_and_barrier(self, tick_clock, wait_clock):
            clock = tick_clock.global_clock
            allocated = wait_clock.sems.allocated()
            for proc, tick in enumerate(clock):
                if tick > 0 and proc in allocated:
                    n = self.nc.sync.nop(nofuse=True, hint="tailwait")
                    n.wait_op(allocated[proc], tick, "sem-ge")
            self.nc.sync.drain()
            self.nc.all_engine_barrier()
            assert self.sems is not None
            popped = self.nc._tile_sem_poison_stack.pop()
            assert popped is self._sem_poison
            self.nc.clear_and_free_semaphores(list(self.sems.allocated().values()))
            self.nc.all_engine_barrier()

    return _TC(nc)


def _build_program(k_lists, variant="full"):
    """k_lists: per sub-tile s, list over rounds r of k_{r,s} (number of
    suffix blocks the round covers). k_lists[s][0] == BPS always."""
    import concourse.bass as bass
    import concourse.mybir as mybir

    do_gather = variant in ("full", "gather")
    do_conv = variant in ("full", "conv")

    f32 = mybir.dt.float32
    bf16 = mybir.dt.bfloat16
    AF = mybir.ActivationFunctionType
    ALU = mybir.AluOpType

    gdt = bf16 if GATHER_BF16 else f32
    nc = bass.Bass()
    g_d = {}
    for s in range(NSUB):
        for r, k in enumerate(k_lists[s]):
            g_d[(s, r)] = nc.dram_tensor(
                f"g{s}_{r}", [128, k * HW], gdt, kind="ExternalInput")
    fown_d = nc.dram_tensor("fown", [64, NB * HW], bf16, kind="ExternalInput")
    w1_d = nc.dram_tensor("w1bd", [128, 9 * 128], bf16, kind="ExternalInput")
    w1sp_d = nc.dram_tensor("w1sp", [128, 3 * 128], bf16, kind="ExternalInput")
    w1sq_d = nc.dram_tensor("w1sq", [128, 3 * 128], bf16, kind="ExternalInput")
    w2_d = nc.dram_tensor("w2bd", [128, 9 * 128], bf16, kind="ExternalInput")
    w3_d = nc.dram_tensor("w3bd", [128, 9 * 64], bf16, kind="ExternalInput")
    y_d = nc.dram_tensor("y", [64, NB * HW], f32, kind="ExternalOutput")

    def valid(ap_grid):
        # [p, GRID] tile AP -> [p, 32, 32] interior view of the 34x34 grid
        return ap_grid.rearrange("p (h w) -> p h w", w=WP)[:, 1:H + 1, 1:W + 1]

    def tap_view(ap_grid, k, dy, dx, h0):
        g3 = ap_grid.rearrange("p (h w) -> p h w", w=WP)
        return g3[0:k, h0 + dy:h0 + dy + 16, dx:dx + W]

    tc = _make_tile_context(nc)
    with tc:
        with (tc.tile_pool(name="cw", bufs=1) as cw,
              tc.tile_pool(name="x1pnp", bufs=3) as x1pnp,
              tc.tile_pool(name="x1sp", bufs=3) as x1sp,
              tc.tile_pool(name="fop", bufs=3) as fop,
              tc.tile_pool(name="x2p", bufs=3) as x2p,
              tc.tile_pool(name="x3p", bufs=3) as x3p,
              tc.tile_pool(name="r2p", bufs=3) as r2p,
              tc.tile_pool(name="r2cp", bufs=3) as r2cp,
              tc.tile_pool(name="osbp", bufs=3) as osbp,
              tc.tile_pool(name="psp", bufs=4, space="PSUM") as psp):
            w1_t = cw.tile([128, 9 * 128], bf16)
            nc.sync.dma_start(out=w1_t[:], in_=w1_d[:])
            w1sp_t = cw.tile([128, 3 * 128], bf16)
            nc.sync.dma_start(out=w1sp_t[:], in_=w1sp_d[:])
            w1sq_t = cw.tile([128, 3 * 128], bf16)
            nc.sync.dma_start(out=w1sq_t[:], in_=w1sq_d[:])
            w2_t = cw.tile([128, 9 * 128], bf16)
            nc.sync.dma_start(out=w2_t[:], in_=w2_d[:])
            w3_t = cw.tile([128, 9 * 64], bf16)
            nc.sync.dma_start(out=w3_t[:], in_=w3_d[:])

            # pooled accumulator sub-tiles, f32
            A = [cw.tile([128, SUBS[s] * HW], f32, tag=f"A{s}", name=f"A{s}")
                 for s in range(NSUB)]
            if do_gather:
                # round 0: bulk bypass init; rounds >=1: per-block accum DMAs
                # (CCE accumulate breaks on >2-dim APs / >8KB runs)
                for s in range(NSUB):
                    for r, k in enumerate(k_lists[s]):
                        if r == 0:
                            nc.gpsimd.dma_start(out=A[s][:], in_=g_d[(s, 0)][:])
                            continue
                        for jj in range(k):
                            j = (SUBS[s] - k) + jj
                            nc.gpsimd.dma_start(
                                out=A[s][:, j * HW:(j + 1) * HW],
                                in_=g_d[(s, r)][:, jj * HW:(jj + 1) * HW],
                                accum_op=ALU.add)
            else:
                for s in range(NSUB):
                    nc.vector.memset(A[s][:], 0.0)

            memset_count = {}

            def fresh_grid(pool, name, parts=128):
                t = pool.tile([parts, GRID], bf16, tag=name)
                c = memset_count.get(name, 0)
                if c < 3:  # pool bufs
                    nc.vector.memset(t[:], 0.0)
                    memset_count[name] = c + 1
                return t

            def sub_of(g):
                for s in range(NSUB - 1, -1, -1):
                    if g >= SUB0[s]:
                        return s, g - SUB0[s]
                raise AssertionError(g)

            def conv1(g):
                s, j = sub_of(g)
                fo = fop.tile([64, HW], bf16, tag="fo")
                nc.sync.dma_start(out=fo[:], in_=fown_d[:, g * HW:(g + 1) * HW])
                # self grid + column-shifted replica in partitions 64..127:
                # x1s[64+p, h, w] == x1s[p, h, w+1]. Issued before the x1pn
                # copy so the DVE isn't blocked on the gather chain.
                x1s = fresh_grid(x1sp, "x1s")
                fov = fo[:].rearrange("p (h w) -> p h w", w=W)
                nc.vector.tensor_copy(out=valid(x1s[0:64]), in_=fov)
                x1sg = x1s[:].rearrange("p (h w) -> p h w", w=WP)
                nc.vector.tensor_copy(out=x1sg[64:128, 1:H + 1, 0:W], in_=fov)
                x1 = fresh_grid(x1pnp, "x1pn")
                nc.vector.tensor_copy(
                    out=valid(x1[:]),
                    in_=A[s][:, j * HW:(j + 1) * HW].rearrange(
                        "p (h w) -> p h w", w=W))
                # self part FIRST: depends only on the fown DMA, so the PE can
                # start before the pooled gather chain completes
                ps1 = psp.tile([128, HW], f32, tag="ps")
                ps1v = ps1[:].rearrange("p (h w) -> p h w", w=W)
                x1sv = x1s[:].rearrange("p (h w) -> p h w", w=WP)
                for dy in range(3):
                    for h0 in (0, 16):
                        # taps (dy,0)+(dy,1) in one K=128 matmul via replica
                        nc.tensor.matmul(
                            out=ps1v[:, h0:h0 + 16, :],
                            lhsT=w1sp_t[:, 128 * dy:128 * dy + 128],
                            rhs=x1sv[0:128, h0 + dy:h0 + dy + 16, 0:W],
                            start=(dy == 0), stop=False)
                    for h0 in (0, 16):
                        # tap (dy,2): replica shifted once more
                        nc.tensor.matmul(
                            out=ps1v[:, h0:h0 + 16, :],
                            lhsT=w1sq_t[64:128, 128 * dy:128 * dy + 128],
                            rhs=x1sv[64:128, h0 + dy:h0 + dy + 16, 1:W + 1],
                            start=False, stop=False)
                for t in range(9):
                    for h0 in (0, 16):
                        nc.tensor.matmul(
                            out=ps1v[:, h0:h0 + 16, :],
                            lhsT=w1_t[:, 128 * t:128 * t + 128],
                            rhs=tap_view(x1[:], 128, t // 3, t % 3, h0),
                            start=False, stop=(t == 8))
                r2a = r2p.tile([128, HW], bf16, tag="r2")
                nc.scalar.activation(out=r2a[:], in_=ps1[:], func=AF.Relu,
                                     scale=-0.9)
                x2 = fresh_grid(x2p, "x2")
                nc.vector.tensor_tensor(
                    out=valid(x2[:]),
                    in0=ps1[:].rearrange("p (h w) -> p h w", w=W),
                    in1=r2a[:].rearrange("p (h w) -> p h w", w=W),
                    op=ALU.add)
                return x2

            def conv2(x2):
                ps2 = psp.tile([128, HW], f32, tag="ps")
                ps2v = ps2[:].rearrange("p (h w) -> p h w", w=W)
                for t in range(9):
                    for h0 in (0, 16):
                        nc.tensor.matmul(
                            out=ps2v[:, h0:h0 + 16, :],
                            lhsT=w2_t[:, 128 * t:128 * t + 128],
                            rhs=tap_view(x2[:], 128, t // 3, t % 3, h0),
                            start=(t == 0), stop=(t == 8))
                r2b = r2p.tile([128, HW], bf16, tag="r2")
                nc.scalar.activation(out=r2b[:], in_=ps2[:], func=AF.Relu,
                                     scale=-0.9)
                x3 = fresh_grid(x3p, "x3")
                nc.vector.tensor_tensor(
                    out=valid(x3[:]),
                    in0=ps2[:].rearrange("p (h w) -> p h w", w=W),
                    in1=r2b[:].rearrange("p (h w) -> p h w", w=W),
                    op=ALU.add)
                return x3

            def conv3(x3, g):
                ps3 = psp.tile([128, HW], f32, tag="ps")
                ps3v = ps3[:].rearrange("p (h w) -> p h w", w=W)
                for t in range(9):
                    for h0 in (0, 16):
                        nc.tensor.matmul(
                            out=ps3v[0:64, h0:h0 + 16, :],
                            lhsT=w3_t[:, 64 * t:64 * t + 64],
                            rhs=tap_view(x3[:], 128, t // 3, t % 3, h0),
                            start=(t == 0), stop=(t == 8))
                r2c = r2cp.tile([64, HW], bf16, tag="r2c")
                nc.scalar.activation(out=r2c[:], in_=ps3[0:64, :], func=AF.Relu,
                                     scale=-0.9)
                osb = osbp.tile([64, HW], f32, tag="osb")
                nc.vector.tensor_tensor(out=osb[:], in0=ps3[0:64, :],
                                        in1=r2c[:], op=ALU.add)
                nc.sync.dma_start(out=y_d[:, g * HW:(g + 1) * HW], in_=osb[:])

            # software pipeline: pairs of bundles, layer-interleaved
            if do_conv:
                for g0 in range(0, NB, 2):
                    x2a = conv1(g0)
                    x2b = conv1(g0 + 1)
                    x3a = conv2(x2a)
                    x3b = conv2(x2b)
                    conv3(x3a, g0)
                    conv3(x3b, g0 + 1)
            else:
                # dump pooled accumulator (partitions 0:64) for inspection
                for g in range(NB):
                    s, j = sub_of(g)
                    nc.sync.dma_start(
                        out=y_d[:, g * HW:(g + 1) * HW],
                        in_=A[s][0:64, j * HW:(j + 1) * HW])
    return nc


def _host_prep(feats, edges, w1, b1, w2, b2, w3, b3):
    import ml_dtypes

    feats = np.ascontiguousarray(np.asarray(feats, dtype=np.float32))
    edges = np.asarray(edges)
    w1 = np.asarray(w1, dtype=np.float32)
    w2 = np.asarray(w2, dtype=np.float32)
    w3 = np.asarray(w3, dtype=np.float32)

    # per-(node, sign) contribution lists (bidirectional)
    contrib = [([], []) for _ in range(N)]
    for s, sg, d in edges.tolist():
        si = 0 if sg > 0 else 1
        contrib[d][si].append(s)
        contrib[s][si].append(d)

    # per-core slot ordering: ascending max-degree so low-degree bundles are
    # gathered first and consumed first by the conv pipeline
    slot2node = []
    for k in range(NCORES):
        nodes = list(range(NPC * k, NPC * (k + 1)))
        nodes.sort(key=lambda n: max(len(contrib[n][0]), len(contrib[n][1])))
        slot2node.append(nodes)

    # block max degree per (core, bundle)
    bmax = np.zeros((NCORES, NB), np.int64)
    for k in range(NCORES):
        for g in range(NB):
            m = 0
            for jm in range(4):
                n = slot2node[k][4 * g + jm]
                m = max(m, len(contrib[n][0]), len(contrib[n][1]))
            bmax[k, g] = m

    # per-sub-tile round coverage, uniform across cores
    k_lists = []
    for s in range(NSUB):
        bs = SUBS[s]
        blk = bmax[:, SUB0[s]:SUB0[s] + bs]  # [cores, bs], ascending per core
        rmax = int(blk.max())
        ks = []
        for r in range(max(rmax, 1)):
            k = int((blk > r).sum(axis=1).max()) if r > 0 else bs
            ks.append(max(k, 1) if r > 0 else bs)
        k_lists.append(ks)

    featsN = feats.reshape(N, C, HW)
    tabN = np.concatenate([featsN, np.zeros((1, C, HW), np.float32)], axis=0)
    tabN_bf = tabN.astype(ml_dtypes.bfloat16)
    tabN_g = tabN_bf if GATHER_BF16 else tabN

    # weight tiles: block-diagonal lhsT layouts
    def bd_tile(wsel, ci_n, co_n):
        # wsel: [co, ci_n, 3, 3]; returns [4*ci_n(? partitions), 9*128-ish]
        t = np.zeros((4 * ci_n, 9 * 4 * co_n), np.float32)
        for jm in range(4):
            for tp in range(9):
                dy, dx = tp // 3, tp % 3
                t[ci_n * jm:ci_n * (jm + 1),
                  4 * co_n * tp + co_n * jm:4 * co_n * tp + co_n * (jm + 1)] = \
                    wsel[:, :, dy, dx].T
        return t.astype(ml_dtypes.bfloat16)

    w1bd = bd_tile(w1[:, C:3 * C], 2 * C, 2 * C)      # [128, 9*128] pos+neg
    w2bd = bd_tile(w2, 2 * C, 2 * C)                  # [128, 9*128]
    w3bd = bd_tile(w3, 2 * C, C)                      # [128, 9*64]
    # conv1-self paired-tap weights: rows 0:64 self ch for dx=0 (w1sp) /
    # zero (w1sq); rows 64:128 replica ch for dx=1 (w1sp) / dx=2 (w1sq)
    w1sp = np.zeros((128, 3 * 128), np.float32)
    w1sq = np.zeros((128, 3 * 128), np.float32)
    for jm in range(4):
        for dy in range(3):
            blk = slice(128 * dy + 32 * jm, 128 * dy + 32 * jm + 32)
            w1sp[16 * jm:16 * jm + 16, blk] = w1[:, 0:C, dy, 0].T
            w1sp[64 + 16 * jm:64 + 16 * jm + 16, blk] = w1[:, 0:C, dy, 1].T
            w1sq[64 + 16 * jm:64 + 16 * jm + 16, blk] = w1[:, 0:C, dy, 2].T
    w1sp = w1sp.astype(ml_dtypes.bfloat16)
    w1sq = w1sq.astype(ml_dtypes.bfloat16)

    in_maps = []
    for k in range(NCORES):
        m = {"w1bd": w1bd, "w1sp": w1sp, "w1sq": w1sq,
             "w2bd": w2bd, "w3bd": w3bd}
        nodes_k = np.array(slot2node[k]).reshape(NB, 4)  # [jM, jm]
        # fown: [64, NB*HW] partitions p=16*jm+c, free = jM*HW + px
        fo = tabN_bf[nodes_k]                  # [jM, jm, C, HW]
        m["fown"] = np.ascontiguousarray(
            fo.transpose(1, 2, 0, 3).reshape(64, NB * HW))
        # gather tables
        for s in range(NSUB):
            for r, kk in enumerate(k_lists[s]):
                # blocks covered: suffix of kk blocks within the sub-tile
                srcs = np.full((8, kk), N, np.int64)  # default: zero row
                for jj in range(kk):
                    g = SUB0[s] + (SUBS[s] - kk) + jj
                    for jm in range(4):
                        n = slot2node[k][4 * g + jm]
                        for sg in range(2):
                            lst = contrib[n][sg]
                            if r < len(lst):
                                srcs[2 * jm + sg, jj] = lst[r]
                arr = tabN_g[srcs]             # [8, kk, C, HW]
                m[f"g{s}_{r}"] = np.ascontiguousarray(
                    arr.transpose(0, 2, 1, 3).reshape(128, kk * HW))
        in_maps.append(m)
    return in_maps, slot2node, tuple(tuple(ks) for ks in k_lists)


def kernel(feats, edges, w1, b1, w2, b2, w3, b3):
    from concourse.bass_utils import run_bass_kernel_spmd

    with_bias = bool(np.any(np.asarray(b1)) or np.any(np.asarray(b2))
                     or np.any(np.asarray(b3)))
    assert not with_bias, "nonzero conv biases not implemented"

    in_maps, slot2node, k_key = _host_prep(
        feats, edges, w1, b1, w2, b2, w3, b3)

    nc = _prog_cache.get(k_key)
    if nc is None:
        nc = _build_program([list(ks) for ks in k_key])
        _prog_cache[k_key] = nc

    import os
    trace = bool(os.environ.get("KERNEL_TRACE"))
    res = run_bass_kernel_spmd(nc, in_maps, core_ids=list(range(NCORES)),
                               trace=trace)
    if trace:
        global last_results
        last_results = res

    out = np.empty((N, C, H, W), np.float32)
    for k in range(NCORES):
        yk = res.results[k]["y"].reshape(4, C, NB, HW)  # [jm, c, jM, px]
        for g in range(NB):
            for jm in range(4):
                n = slot2node[k][4 * g + jm]
                out[n] = yk[jm, :, g, :].reshape(C, H, W)
    return out
